# revision 4
# baseline (speedup 1.0000x reference)
"""Trainium2 Bass kernel for DecoderAttnRNN — v3 pipelined.

Sharding: 8 cores = 4 batch-groups x 2 vocab-halves (BL=16, VL=16000).

Single fused pipeline in t-major layout (z column = t*BL + b):
  prologue: ordered DMAs, embedding gather, x@W_ih+bias precompute (xwT)
  9 blocks of 8 LSTM steps; attention in sub-blocks [8,8,16,16,16,8];
  GEMM (column-group x m-tile) pairs interleaved between LSTM steps as
  soon as (a) the wlin group slice has arrived via DMA and (b) the
  m-tile's h/ctx columns are ready.  Gates matmuls accumulate onto PSUM
  preloaded with xw+bias.  Outputs evacuated to bf16 and scatter-DMA'd
  to b-major rows; bias and f32 upcast happen on host.

PSUM banks: gemm 2x2 + gates 1 + attn sc 2 + attn misc 1 = 8.
"""

import numpy as np
import ml_dtypes

import concourse.bass as bass
import concourse.mybir as mybir
import concourse.tile as tile
from concourse import bacc
from concourse.bass_utils import run_bass_kernel_spmd
from concourse.masks import make_identity

B, T, S, E, H, V = 64, 72, 72, 128, 256, 32000
NCORES = 8
NBG = 4                   # batch groups
NVH = 2                   # vocab halves
BL = B // NBG             # 16 batches per core
BT = BL * T               # 1152 (t-major: flat col = t*BL + b)
VL = V // NVH             # 16000 vocab cols per core
G4H = 4 * H               # 1024
NCH = G4H // 128          # 8 gate chunks of 128
NMT = BT // 128           # 9 m-tiles (each = 8 timesteps x 16 batches)
TPB = T // NMT            # 8 timesteps per block

NG = 32                   # GEMM column groups of 500
GCOL = VL // NG           # 500
# group g's weights arrive (DMA order) around block ARRIVE[g]
ARRIVE = [min(g // 4, 7) for g in range(NG)]
# attention sub-blocks (t0, t1, done_at_block)
SBLOCKS = [(0, 8, 0), (8, 16, 1), (16, 32, 3), (32, 48, 5), (48, 64, 7), (64, 72, 8)]
# m-tile m is GEMM-ready at end of block MREADY[m]
MREADY = [0, 1, 3, 3, 5, 5, 7, 7, 8]

f32 = mybir.dt.float32
bf16 = mybir.dt.bfloat16
i32 = mybir.dt.int32

_CACHE = {}


def _build():
    nc = bacc.Bacc(None, target_bir_lowering=False)

    tok_d = nc.declare_dram_parameter("tok", [BT, 1], i32, isOutput=False)
    emb_d = nc.declare_dram_parameter("emb", [V, E], f32, isOutput=False)
    enc_d = nc.declare_dram_parameter("enc", [S, BL, H], bf16, isOutput=False)
    encT_d = nc.declare_dram_parameter("encT", [2, 128, BL, S], bf16, isOutput=False)
    h0T_d = nc.declare_dram_parameter("h0T", [128, 2, BL], f32, isOutput=False)
    c0T_d = nc.declare_dram_parameter("c0T", [128, 2, BL], f32, isOutput=False)
    lens_d = nc.declare_dram_parameter("lens", [BL], i32, isOutput=False)
    biasT_d = nc.declare_dram_parameter("biasT", [128, NCH], f32, isOutput=False)
    wihT_d = nc.declare_dram_parameter("wihT", [E, G4H], bf16, isOutput=False)
    whhT_d = nc.declare_dram_parameter("whhT", [2, 128, G4H], bf16, isOutput=False)
    wlinT_d = nc.declare_dram_parameter("wlinT", [4, 128, VL], bf16, isOutput=False)
    out_d = nc.declare_dram_parameter("logits", [BT, VL], bf16, isOutput=True)

    with tile.TileContext(nc) as tc:
        with (
            tc.tile_pool(name="persist", bufs=1) as pp,
            tc.tile_pool(name="lstm_sb", bufs=2) as lsp,
            tc.tile_pool(name="out_sb", bufs=3) as osp,
        ):
            # ---------------- prologue DMAs, ordered by need ----------------
            # Pool queue: tok + small state, then gathers (phase 0), then
            # encoder tensors, then output DMAs.
            tok_sb = pp.tile([128, NMT], i32)
            nc.gpsimd.dma_start(
                out=tok_sb[:],
                in_=bass.AP(tensor=tok_d, offset=0, ap=[[1, 128], [128, NMT]]),
            )
            h0f = pp.tile([128, 2, BL], f32)
            nc.gpsimd.dma_start(out=h0f[:], in_=h0T_d[:])
            cT = pp.tile([128, 2, BL], f32)
            nc.gpsimd.dma_start(out=cT[:], in_=c0T_d[:])
            lens_i = pp.tile([S, BL], i32)
            lens_bcast = bass.AP(tensor=lens_d, offset=0, ap=[[0, S], [1, BL]])
            nc.gpsimd.dma_start(out=lens_i[:], in_=lens_bcast)
            biasT_sb = pp.tile([128, NCH], f32)
            nc.gpsimd.dma_start(out=biasT_sb[:], in_=biasT_d[:])
            # Act queue: xw weights first (needed ~5us), then recurrence.
            wih_sb = pp.tile([128, G4H], bf16)
            nc.scalar.dma_start(out=wih_sb[:], in_=wihT_d[:])
            whh_sb = pp.tile([128, 2, G4H], bf16)
            for k in range(2):
                nc.scalar.dma_start(out=whh_sb[:, k, :], in_=whhT_d[k])

            # SP queue gets even wlin groups, Act queue the odd ones.
            wlin_sb = pp.tile([128, 4, VL], bf16)
            for g in range(NG):
                n0 = g * GCOL
                src = bass.AP(
                    tensor=wlinT_d,
                    offset=n0,
                    ap=[[VL, 128], [128 * VL, 4], [1, GCOL]],
                )
                eng = nc.sync if g % 2 == 0 else nc.scalar
                eng.dma_start(out=wlin_sb[:, :, n0 : n0 + GCOL], in_=src)

            # ---------------- persistent compute tiles ----------------
            xwT = pp.tile([128, T, NCH, BL], bf16)     # x@W_ih.T + bias (t-major)
            z01 = pp.tile([128, 2, T, BL], bf16)       # h features
            z23 = pp.tile([128, 2, T, BL], bf16)       # ctx features

            h_init = pp.tile([128, 2, BL], bf16)
            nc.vector.tensor_copy(out=h_init[:], in_=h0f[:])

            ident = pp.tile([128, 128], f32)
            make_identity(nc, ident[:])
            ones_col = pp.tile([S, 1], bf16)
            nc.vector.memset(ones_col[:], 1.0)
            ones_row = pp.tile([1, 128], f32)
            nc.vector.memset(ones_row[:], 1.0)

            # attention mask: mask01[s, b] = 1.0 if s < len_b else 0.0
            lens_f = pp.tile([S, BL], f32)
            nc.vector.tensor_copy(out=lens_f[:], in_=lens_i[:])
            iota_i = pp.tile([S, 1], i32)
            nc.gpsimd.iota(iota_i[:], [[1, 1]], base=0, channel_multiplier=1)
            iota_f = pp.tile([S, 1], f32)
            nc.vector.tensor_copy(out=iota_f[:], in_=iota_i[:])
            mask01 = pp.tile([S, BL], f32)
            nc.vector.tensor_scalar(
                out=mask01[:], in0=lens_f[:], scalar1=iota_f[:], scalar2=None,
                op0=mybir.AluOpType.is_gt,
            )

            expsc = pp.tile([S, BL, 16], bf16)         # exp-scores of active sblock
            recip_sb = pp.tile([1, BL * 16], f32)
            bc_sb = pp.tile([128, BL * 16], bf16)
            encT_sb = pp.tile([128, 2, BL, S], bf16)
            enc_sb = pp.tile([S, BL, H], bf16)

            # ---------------- phase 0: gather + xw precompute ----------------
            x_allT = pp.tile([128, BT], bf16)
            with (
                tc.tile_pool(name="p0", bufs=2) as wp,
                tc.tile_pool(name="p0ps", bufs=2, space="PSUM") as psp,
            ):
                for j in range(NMT):
                    x_t = wp.tile([128, E], f32, tag="x")
                    nc.gpsimd.indirect_dma_start(
                        out=x_t[:],
                        out_offset=None,
                        in_=emb_d[:],
                        in_offset=bass.IndirectOffsetOnAxis(
                            ap=tok_sb[:, j : j + 1], axis=0
                        ),
                    )
                    ps_t = psp.tile([128, 128], f32, tag="pst")
                    nc.tensor.transpose(out=ps_t[:], in_=x_t[:], identity=ident[:])
                    nc.vector.tensor_copy(
                        out=x_allT[:, j * 128 : (j + 1) * 128], in_=ps_t[:]
                    )

                # xw in t-segments so block 0 unblocks early
                for si, (s0, sn) in enumerate([(0, 512), (512, 512), (1024, 128)]):
                    for c in range(NCH):
                        ps_xw = psp.tile([128, 512], f32, tag="psxw")
                        nc.tensor.matmul(
                            ps_xw[:, :sn],
                            wih_sb[:, c * 128 : (c + 1) * 128],
                            x_allT[:, s0 : s0 + sn],
                            start=True,
                            stop=True,
                        )
                        if si == 0:
                            nc.vector.tensor_scalar(
                                out=xwT[:, s0 // BL : (s0 + sn) // BL, c, :],
                                in0=ps_xw[:, :sn].rearrange(
                                    "p (t b) -> p t b", b=BL
                                ),
                                scalar1=biasT_sb[:, c : c + 1],
                                scalar2=None,
                                op0=mybir.AluOpType.add,
                            )
                        else:
                            nc.scalar.activation(
                                out=xwT[:, s0 // BL : (s0 + sn) // BL, c, :],
                                in_=ps_xw[:, :sn].rearrange(
                                    "p (t b) -> p t b", b=BL
                                ),
                                func=mybir.ActivationFunctionType.Identity,
                                bias=biasT_sb[:, c : c + 1],
                            )

            # encoder tensors on Pool queue, behind the gathers (needed at
            # the first attention sblock, ~25-30us in)
            for k in range(2):
                nc.gpsimd.dma_start(out=encT_sb[:, k], in_=encT_d[k])
            nc.gpsimd.dma_start(out=enc_sb[:], in_=enc_d[:])

            # ---------------- pipelined main loop ----------------
            inv_sqrt_h = float(1.0 / np.sqrt(H))
            zt = [
                z01[:, 0].rearrange("p t b -> p (t b)"),
                z01[:, 1].rearrange("p t b -> p (t b)"),
                z23[:, 0].rearrange("p t b -> p (t b)"),
                z23[:, 1].rearrange("p t b -> p (t b)"),
            ]

            with (
                tc.tile_pool(name="gates_ps", bufs=1, space="PSUM") as gps,
                tc.tile_pool(name="attn_ps", bufs=2, space="PSUM") as aps,
                tc.tile_pool(name="attn_ps1", bufs=1, space="PSUM") as aps1,
                tc.tile_pool(name="gemm_ps", bufs=3, space="PSUM") as ops,
            ):
                arrived = []     # weight groups available in SBUF
                ready_m = []     # m-tiles whose h+ctx columns are complete
                pendq = []       # (g, m) pairs not yet emitted
                state = {"pair": 0}

                def lstm_step(t):
                    # gates psum preloaded with xw+bias; matmuls accumulate
                    ps_g = gps.tile([128, NCH, BL], f32, tag="psg")
                    nc.vector.tensor_copy(out=ps_g[:], in_=xwT[:, t])
                    for c in range(NCH):
                        for k in range(2):
                            rhs = (
                                h_init[:, k, :]
                                if t == 0
                                else z01[:, k, t - 1, :]
                            )
                            nc.tensor.matmul(
                                ps_g[:, c, :],
                                whh_sb[:, k, c * 128 : (c + 1) * 128],
                                rhs,
                                start=False,
                                stop=(c == NCH - 1 and k == 1),
                                skip_group_check=True,
                            )
                    # chunks: 0-1=i, 2-3=f, 4-5=o, 6-7=g (host-permuted)
                    sig = lsp.tile([128, 6, BL], f32, tag="sig")
                    nc.scalar.activation(
                        out=sig[:], in_=ps_g[:, 0:6],
                        func=mybir.ActivationFunctionType.Sigmoid,
                    )
                    tg = lsp.tile([128, 2, BL], f32, tag="tg")
                    nc.scalar.activation(
                        out=tg[:], in_=ps_g[:, 6:8],
                        func=mybir.ActivationFunctionType.Tanh,
                    )
                    # c = sig(f)*c + sig(i)*tanh(g)
                    nc.vector.tensor_tensor(
                        out=cT[:], in0=sig[:, 2:4], in1=cT[:],
                        op=mybir.AluOpType.mult,
                    )
                    ig = lsp.tile([128, 2, BL], f32, tag="ig")
                    nc.gpsimd.tensor_tensor(
                        out=ig[:], in0=sig[:, 0:2], in1=tg[:],
                        op=mybir.AluOpType.mult,
                    )
                    nc.vector.tensor_tensor(
                        out=cT[:], in0=cT[:], in1=ig[:], op=mybir.AluOpType.add
                    )
                    th = lsp.tile([128, 2, BL], f32, tag="th")
                    nc.scalar.activation(
                        out=th[:], in_=cT[:],
                        func=mybir.ActivationFunctionType.Tanh,
                    )
                    nc.gpsimd.tensor_tensor(
                        out=z01[:, :, t, :], in0=sig[:, 4:6], in1=th[:],
                        op=mybir.AluOpType.mult,
                    )

                def attention(t0, t1):
                    sb = t1 - t0
                    # all scores of the sblock into one psum tile, one Exp
                    ps_sall = aps1.tile([S, BL * 16], f32, tag="sall")
                    for b in range(BL):
                        for k in range(2):
                            nc.tensor.matmul(
                                ps_sall[:, b * sb : b * sb + sb],
                                encT_sb[:, k, b, :],
                                z01[:, k, t0:t1, b],
                                start=(k == 0),
                                stop=(k == 1),
                            )
                    nc.scalar.activation(
                        out=expsc[:, :, :sb],
                        in_=ps_sall[:, : BL * sb].rearrange(
                            "s (b t) -> s b t", b=BL
                        ),
                        func=mybir.ActivationFunctionType.Exp,
                        scale=inv_sqrt_h,
                    )
                    for b in range(BL):
                        nc.gpsimd.tensor_scalar(
                            out=expsc[:, b, :sb], in0=expsc[:, b, :sb],
                            scalar1=mask01[:, b : b + 1], scalar2=None,
                            op0=mybir.AluOpType.mult,
                        )
                    # softmax denominators for all (b, t'): [1, BL*sb]
                    ps_d = aps1.tile([1, BL * 16], f32, tag="misc")
                    nc.tensor.matmul(
                        ps_d[:, : BL * sb], ones_col[:], expsc[:, :, :sb],
                        start=True, stop=True,
                    )
                    nc.vector.reciprocal(
                        out=recip_sb[:, : BL * sb], in_=ps_d[:, : BL * sb]
                    )
                    # broadcast reciprocals to all 128 partitions
                    for ch in range((BL * sb + 127) // 128):
                        c0 = ch * 128
                        cn = min(128, BL * sb - c0)
                        ps_bc = aps1.tile([128, 128], f32, tag="misc")
                        nc.tensor.matmul(
                            ps_bc[:, :cn], ones_row[:],
                            recip_sb[:, c0 : c0 + cn],
                            start=True, stop=True,
                        )
                        nc.vector.tensor_copy(
                            out=bc_sb[:, c0 : c0 + cn], in_=ps_bc[:, :cn]
                        )
                    # ctx per (b, j), normalized into z23
                    for b in range(BL):
                        for j in range(2):
                            ps_c = aps.tile([128, 16], f32, tag="sc")
                            nc.tensor.matmul(
                                ps_c[:, :sb],
                                enc_sb[:, b, j * 128 : (j + 1) * 128],
                                expsc[:, b, :sb],
                                start=True, stop=True,
                            )
                            nc.vector.tensor_tensor(
                                out=z23[:, j, t0:t1, b],
                                in0=ps_c[:, :sb],
                                in1=bc_sb[:, b * sb : (b + 1) * sb],
                                op=mybir.AluOpType.mult,
                            )

                def gemm_pair(g, m):
                    n0 = g * GCOL
                    ps_o = ops.tile([128, 512], f32, tag="po")
                    for k in range(4):
                        nc.tensor.matmul(
                            ps_o[:, :GCOL],
                            zt[k][:, m * 128 : (m + 1) * 128],
                            wlin_sb[:, k, n0 : n0 + GCOL],
                            start=(k == 0),
                            stop=(k == 3),
                        )
                    o_sb = osp.tile([128, GCOL], bf16, tag="osb")
                    pi = state["pair"]
                    if pi % 2 == 0:
                        nc.vector.tensor_copy(out=o_sb[:], in_=ps_o[:, :GCOL])
                    else:
                        nc.scalar.copy(out=o_sb[:], in_=ps_o[:, :GCOL])
                    # scatter to b-major rows: row = b*T + 8m + t_loc
                    dst = bass.AP(
                        tensor=out_d,
                        offset=(TPB * m) * VL + n0,
                        ap=[[VL, TPB], [T * VL, BL], [1, GCOL]],
                    )
                    dma_eng = [nc.gpsimd, nc.sync][pi % 2]
                    dma_eng.dma_start(out=dst, in_=o_sb[:])
                    state["pair"] = pi + 1

                def drain(nmax):
                    n = 0
                    while pendq and n < nmax:
                        gemm_pair(*pendq.pop(0))
                        n += 1

                sb_iter = iter(SBLOCKS)
                next_sb = next(sb_iter)
                for i in range(NMT):
                    for t in range(i * TPB, (i + 1) * TPB):
                        lstm_step(t)
                        drain(4 + (len(pendq) > 12) * 2 + (len(pendq) > 24) * 2)
                    while next_sb is not None and next_sb[2] == i:
                        attention(next_sb[0], next_sb[1])
                        next_sb = next(sb_iter, None)
                    new_m = [m for m in range(NMT) if MREADY[m] == i]
                    new_g = [g for g in range(NG) if ARRIVE[g] == i]
                    for g in new_g:
                        for m in ready_m:
                            pendq.append((g, m))
                    for m in new_m:
                        for g in arrived + new_g:
                            pendq.append((g, m))
                    arrived += new_g
                    ready_m += new_m
                drain(10**9)

    nc.compile()
    return nc


def _prep_inputs(inputs):
    bf = ml_dtypes.bfloat16
    target = np.asarray(inputs["target_tensor"])
    enc = np.asarray(inputs["encoder_outputs"], dtype=np.float32)
    lens = np.asarray(inputs["encoder_seq_lens"])
    h0 = np.asarray(inputs["h0"], dtype=np.float32)
    c0 = np.asarray(inputs["c0"], dtype=np.float32)
    emb = np.ascontiguousarray(np.asarray(inputs["emb"], dtype=np.float32))
    W_ih = np.asarray(inputs["W_ih"], dtype=np.float32)
    W_hh = np.asarray(inputs["W_hh"], dtype=np.float32)
    bias = (
        np.asarray(inputs["b_ih"], dtype=np.float32)
        + np.asarray(inputs["b_hh"], dtype=np.float32)
    )
    # permute gate order (i, f, g, o) -> (i, f, o, g)
    perm = np.concatenate(
        [np.arange(0, 2 * H), np.arange(3 * H, 4 * H), np.arange(2 * H, 3 * H)]
    )
    W_ih = W_ih[perm]
    W_hh = W_hh[perm]
    bias = bias[perm]
    W_lin = np.asarray(inputs["W_lin"], dtype=np.float32)

    wihT = np.ascontiguousarray(W_ih.T.astype(bf))                # (E, 4H)
    whhT = np.ascontiguousarray(W_hh.T.reshape(2, 128, G4H).astype(bf))
    biasT = np.ascontiguousarray(bias.reshape(NCH, 128).T)        # (128, NCH)
    wlinT_full = W_lin.T.astype(bf)                               # (512, V)

    in_maps = []
    for i in range(NCORES):
        bg = i % NBG
        vh = i // NBG
        sl = slice(bg * BL, (bg + 1) * BL)
        vsl = slice(vh * VL, (vh + 1) * VL)
        tok = np.ascontiguousarray(
            target[sl].T.reshape(BT, 1).astype(np.int32)
        )  # t-major
        enc_i = enc[sl]                                           # (BL, S, H)
        enc_sbh = np.ascontiguousarray(enc_i.transpose(1, 0, 2).astype(bf))
        encT = np.ascontiguousarray(
            enc_i.transpose(2, 0, 1).reshape(2, 128, BL, S).astype(bf)
        )
        h0T = np.ascontiguousarray(h0[sl].T.reshape(2, 128, BL).transpose(1, 0, 2))
        c0T = np.ascontiguousarray(c0[sl].T.reshape(2, 128, BL).transpose(1, 0, 2))
        wlinT = np.ascontiguousarray(wlinT_full[:, vsl].reshape(4, 128, VL))
        in_maps.append(
            {
                "tok": tok,
                "emb": emb,
                "enc": enc_sbh,
                "encT": encT,
                "h0T": h0T,
                "c0T": c0T,
                "lens": np.ascontiguousarray(lens[sl].astype(np.int32)),
                "biasT": biasT,
                "wihT": wihT,
                "whhT": whhT,
                "wlinT": wlinT,
            }
        )
    return in_maps


LAST_RESULTS = None


def _install_ntff_shim():
    import sys
    import types

    try:
        from antenv.axon_hooks import get_axon_ntff_profile_hook  # noqa: F401

        return
    except ImportError:
        pass
    try:
        from trn_agent_boot.trn_boot import _ntff_profile_via_ctypes

        hook = _ntff_profile_via_ctypes("/opt/axon/libaxon_pjrt.so")
        m = types.ModuleType("antenv.axon_hooks")
        m.get_axon_ntff_profile_hook = lambda: hook
        m.set_axon_ntff_profile_hook = lambda h: None
        sys.modules["antenv.axon_hooks"] = m
    except Exception:
        pass


def kernel(**inputs):
    global LAST_RESULTS
    _install_ntff_shim()
    if "nc" not in _CACHE:
        _CACHE["nc"] = _build()
    nc = _CACHE["nc"]
    in_maps = _prep_inputs(inputs)
    res = run_bass_kernel_spmd(nc, in_maps, core_ids=list(range(NCORES)))
    LAST_RESULTS = res
    b_lin = np.asarray(inputs["b_lin"], dtype=np.float32)
    out = np.empty((B, T, V), dtype=np.float32)
    for i in range(NCORES):
        bg = i % NBG
        vh = i // NBG
        vsl = slice(vh * VL, (vh + 1) * VL)
        # logits rows are b-major (row = b*T + t) thanks to the scatter DMA
        out[bg * BL : (bg + 1) * BL, :, vsl] = (
            np.asarray(res.results[i]["logits"]).astype(np.float32).reshape(BL, T, VL)
            + b_lin[vsl]
        )
    return out


# revision 5
# speedup vs baseline: 1.2879x; 1.2879x over previous
"""Trainium2 Bass kernel for DecoderAttnRNN — v3 pipelined.

Sharding: 8 cores = 4 batch-groups x 2 vocab-halves (BL=16, VL=16000).

Single fused pipeline in t-major layout (z column = t*BL + b):
  prologue: ordered DMAs, embedding gather, x@W_ih+bias precompute (xwT)
  9 blocks of 8 LSTM steps; attention in sub-blocks [8,8,16,16,16,8];
  GEMM (column-group x m-tile) pairs interleaved between LSTM steps as
  soon as (a) the wlin group slice has arrived via DMA and (b) the
  m-tile's h/ctx columns are ready.  Gates matmuls accumulate onto PSUM
  preloaded with xw+bias.  Outputs evacuated to bf16 and scatter-DMA'd
  to b-major rows; bias and f32 upcast happen on host.

PSUM banks: gemm 2x2 + gates 1 + attn sc 2 + attn misc 1 = 8.
"""

import numpy as np
import ml_dtypes

import concourse.bass as bass
import concourse.mybir as mybir
import concourse.tile as tile
from concourse import bacc
from concourse.bass_utils import run_bass_kernel_spmd
from concourse.masks import make_identity

B, T, S, E, H, V = 64, 72, 72, 128, 256, 32000
NCORES = 8
NBG = 4                   # batch groups
NVH = 2                   # vocab halves
BL = B // NBG             # 16 batches per core
BT = BL * T               # 1152 (t-major: flat col = t*BL + b)
VL = V // NVH             # 16000 vocab cols per core
G4H = 4 * H               # 1024
NCH = G4H // 128          # 8 gate chunks of 128
NMT = BT // 128           # 9 m-tiles (each = 8 timesteps x 16 batches)
TPB = T // NMT            # 8 timesteps per block

NG = 16                   # GEMM column groups of 1000
GCOL = VL // NG           # 1000
# group g's weights arrive (DMA order) around block ARRIVE[g]
ARRIVE = [0, 0, 0, 1, 1, 2, 2, 3, 3, 4, 4, 5, 5, 6, 6, 7]
# attention sub-blocks (t0, t1, done_at_block)
SBLOCKS = [(0, 8, 0), (8, 16, 1), (16, 32, 3), (32, 48, 5), (48, 64, 7), (64, 72, 8)]
# m-tile m is GEMM-ready at end of block MREADY[m]
MREADY = [0, 1, 3, 3, 5, 5, 7, 7, 8]

f32 = mybir.dt.float32
bf16 = mybir.dt.bfloat16
i32 = mybir.dt.int32

_CACHE = {}


def _build():
    nc = bacc.Bacc(None, target_bir_lowering=False)

    tok_d = nc.declare_dram_parameter("tok", [BT, 1], i32, isOutput=False)
    emb_d = nc.declare_dram_parameter("emb", [V, E], f32, isOutput=False)
    enc_d = nc.declare_dram_parameter("enc", [S, BL, H], bf16, isOutput=False)
    encT_d = nc.declare_dram_parameter("encT", [2, 128, BL, S], bf16, isOutput=False)
    h0T_d = nc.declare_dram_parameter("h0T", [128, 2, BL], f32, isOutput=False)
    c0T_d = nc.declare_dram_parameter("c0T", [128, 2, BL], f32, isOutput=False)
    lens_d = nc.declare_dram_parameter("lens", [BL], i32, isOutput=False)
    biasT_d = nc.declare_dram_parameter("biasT", [128, NCH], f32, isOutput=False)
    wihT_d = nc.declare_dram_parameter("wihT", [E, G4H], bf16, isOutput=False)
    whhT_d = nc.declare_dram_parameter("whhT", [2, 128, G4H], bf16, isOutput=False)
    wlinT_d = nc.declare_dram_parameter("wlinT", [4, 128, VL], bf16, isOutput=False)
    out_d = nc.declare_dram_parameter("logits", [BT, VL], bf16, isOutput=True)

    with tile.TileContext(nc) as tc:
        with (
            tc.tile_pool(name="persist", bufs=1) as pp,
            tc.tile_pool(name="lstm_sb", bufs=2) as lsp,
            tc.tile_pool(name="out_sb", bufs=3) as osp,
        ):
            # ---------------- prologue DMAs, ordered by need ----------------
            # Pool queue: tok + small state, then gathers (phase 0), then
            # encoder tensors, then output DMAs.
            tok_sb = pp.tile([128, NMT], i32)
            nc.gpsimd.dma_start(
                out=tok_sb[:],
                in_=bass.AP(tensor=tok_d, offset=0, ap=[[1, 128], [128, NMT]]),
            )
            h0f = pp.tile([128, 2, BL], f32)
            nc.gpsimd.dma_start(out=h0f[:], in_=h0T_d[:])
            cT = pp.tile([128, 2, BL], f32)
            nc.gpsimd.dma_start(out=cT[:], in_=c0T_d[:])
            lens_i = pp.tile([S, BL], i32)
            lens_bcast = bass.AP(tensor=lens_d, offset=0, ap=[[0, S], [1, BL]])
            nc.gpsimd.dma_start(out=lens_i[:], in_=lens_bcast)
            biasT_sb = pp.tile([128, NCH], f32)
            nc.gpsimd.dma_start(out=biasT_sb[:], in_=biasT_d[:])
            # Act queue: xw weights first (needed ~5us), then recurrence.
            wih_sb = pp.tile([128, G4H], bf16)
            nc.scalar.dma_start(out=wih_sb[:], in_=wihT_d[:])
            whh_sb = pp.tile([128, 2, G4H], bf16)
            for k in range(2):
                nc.scalar.dma_start(out=whh_sb[:, k, :], in_=whhT_d[k])

            # SP queue gets even wlin groups, Act queue the odd ones.
            wlin_sb = pp.tile([128, 4, VL], bf16)
            for g in range(NG):
                n0 = g * GCOL
                src = bass.AP(
                    tensor=wlinT_d,
                    offset=n0,
                    ap=[[VL, 128], [128 * VL, 4], [1, GCOL]],
                )
                eng = nc.sync if g % 2 == 0 else nc.scalar
                eng.dma_start(out=wlin_sb[:, :, n0 : n0 + GCOL], in_=src)

            # ---------------- persistent compute tiles ----------------
            xwT = pp.tile([128, T, NCH, BL], bf16)     # x@W_ih.T + bias (t-major)
            z01 = pp.tile([128, 2, T, BL], bf16)       # h features
            z23 = pp.tile([128, 2, T, BL], bf16)       # ctx features

            h_init = pp.tile([128, 2, BL], bf16)
            nc.vector.tensor_copy(out=h_init[:], in_=h0f[:])

            ident = pp.tile([128, 128], f32)
            make_identity(nc, ident[:])
            ones_col = pp.tile([S, 1], bf16)
            nc.vector.memset(ones_col[:], 1.0)
            ones_row = pp.tile([1, 128], f32)
            nc.vector.memset(ones_row[:], 1.0)

            # attention mask: mask01[s, b] = 1.0 if s < len_b else 0.0
            lens_f = pp.tile([S, BL], f32)
            nc.vector.tensor_copy(out=lens_f[:], in_=lens_i[:])
            iota_i = pp.tile([S, 1], i32)
            nc.gpsimd.iota(iota_i[:], [[1, 1]], base=0, channel_multiplier=1)
            iota_f = pp.tile([S, 1], f32)
            nc.vector.tensor_copy(out=iota_f[:], in_=iota_i[:])
            mask01 = pp.tile([S, BL], f32)
            nc.vector.tensor_scalar(
                out=mask01[:], in0=lens_f[:], scalar1=iota_f[:], scalar2=None,
                op0=mybir.AluOpType.is_gt,
            )

            expsc = pp.tile([S, BL, 16], bf16)         # exp-scores of active sblock
            recip_sb = pp.tile([1, BL * 16], f32)
            bc_sb = pp.tile([128, BL * 16], bf16)
            encT_sb = pp.tile([128, 2, BL, S], bf16)
            enc_sb = pp.tile([S, BL, H], bf16)

            # ---------------- phase 0: gather + xw precompute ----------------
            x_allT = pp.tile([128, BT], bf16)
            with (
                tc.tile_pool(name="p0", bufs=2) as wp,
                tc.tile_pool(name="p0ps", bufs=2, space="PSUM") as psp,
            ):
                for j in range(NMT):
                    x_t = wp.tile([128, E], f32, tag="x")
                    nc.gpsimd.indirect_dma_start(
                        out=x_t[:],
                        out_offset=None,
                        in_=emb_d[:],
                        in_offset=bass.IndirectOffsetOnAxis(
                            ap=tok_sb[:, j : j + 1], axis=0
                        ),
                    )
                    ps_t = psp.tile([128, 128], f32, tag="pst")
                    nc.tensor.transpose(out=ps_t[:], in_=x_t[:], identity=ident[:])
                    nc.vector.tensor_copy(
                        out=x_allT[:, j * 128 : (j + 1) * 128], in_=ps_t[:]
                    )

                # xw in t-segments so block 0 unblocks early
                for si, (s0, sn) in enumerate([(0, 512), (512, 512), (1024, 128)]):
                    for c in range(NCH):
                        ps_xw = psp.tile([128, 512], f32, tag="psxw")
                        nc.tensor.matmul(
                            ps_xw[:, :sn],
                            wih_sb[:, c * 128 : (c + 1) * 128],
                            x_allT[:, s0 : s0 + sn],
                            start=True,
                            stop=True,
                        )
                        if si == 0:
                            nc.vector.tensor_scalar(
                                out=xwT[:, s0 // BL : (s0 + sn) // BL, c, :],
                                in0=ps_xw[:, :sn].rearrange(
                                    "p (t b) -> p t b", b=BL
                                ),
                                scalar1=biasT_sb[:, c : c + 1],
                                scalar2=None,
                                op0=mybir.AluOpType.add,
                            )
                        else:
                            nc.scalar.activation(
                                out=xwT[:, s0 // BL : (s0 + sn) // BL, c, :],
                                in_=ps_xw[:, :sn].rearrange(
                                    "p (t b) -> p t b", b=BL
                                ),
                                func=mybir.ActivationFunctionType.Identity,
                                bias=biasT_sb[:, c : c + 1],
                            )

            # encoder tensors on Pool queue, behind the gathers (needed at
            # the first attention sblock, ~25-30us in)
            for k in range(2):
                nc.gpsimd.dma_start(out=encT_sb[:, k], in_=encT_d[k])
            nc.gpsimd.dma_start(out=enc_sb[:], in_=enc_d[:])

            # ---------------- pipelined main loop ----------------
            inv_sqrt_h = float(1.0 / np.sqrt(H))
            zt = [
                z01[:, 0].rearrange("p t b -> p (t b)"),
                z01[:, 1].rearrange("p t b -> p (t b)"),
                z23[:, 0].rearrange("p t b -> p (t b)"),
                z23[:, 1].rearrange("p t b -> p (t b)"),
            ]

            with (
                tc.tile_pool(name="gates_ps", bufs=1, space="PSUM") as gps,
                tc.tile_pool(name="attn_ps", bufs=2, space="PSUM") as aps,
                tc.tile_pool(name="attn_ps1", bufs=1, space="PSUM") as aps1,
                tc.tile_pool(name="gemm_ps", bufs=2, space="PSUM") as ops,
            ):
                arrived = []     # weight groups available in SBUF
                ready_m = []     # m-tiles whose h+ctx columns are complete
                pendq = []       # (g, m) pairs not yet emitted
                state = {"pair": 0}

                def lstm_step(t):
                    # gates psum preloaded with xw+bias; matmuls accumulate
                    ps_g = gps.tile([128, NCH, BL], f32, tag="psg")
                    nc.vector.tensor_copy(out=ps_g[:], in_=xwT[:, t])
                    for c in range(NCH):
                        for k in range(2):
                            rhs = (
                                h_init[:, k, :]
                                if t == 0
                                else z01[:, k, t - 1, :]
                            )
                            nc.tensor.matmul(
                                ps_g[:, c, :],
                                whh_sb[:, k, c * 128 : (c + 1) * 128],
                                rhs,
                                start=False,
                                stop=(c == NCH - 1 and k == 1),
                                skip_group_check=True,
                            )
                    # chunks: 0-1=i, 2-3=f, 4-5=o, 6-7=g (host-permuted)
                    sig = lsp.tile([128, 6, BL], f32, tag="sig")
                    nc.scalar.activation(
                        out=sig[:], in_=ps_g[:, 0:6],
                        func=mybir.ActivationFunctionType.Sigmoid,
                    )
                    tg = lsp.tile([128, 2, BL], f32, tag="tg")
                    nc.scalar.activation(
                        out=tg[:], in_=ps_g[:, 6:8],
                        func=mybir.ActivationFunctionType.Tanh,
                    )
                    # c = sig(f)*c + sig(i)*tanh(g)
                    nc.vector.tensor_tensor(
                        out=cT[:], in0=sig[:, 2:4], in1=cT[:],
                        op=mybir.AluOpType.mult,
                    )
                    ig = lsp.tile([128, 2, BL], f32, tag="ig")
                    nc.gpsimd.tensor_tensor(
                        out=ig[:], in0=sig[:, 0:2], in1=tg[:],
                        op=mybir.AluOpType.mult,
                    )
                    nc.vector.tensor_tensor(
                        out=cT[:], in0=cT[:], in1=ig[:], op=mybir.AluOpType.add
                    )
                    th = lsp.tile([128, 2, BL], f32, tag="th")
                    nc.scalar.activation(
                        out=th[:], in_=cT[:],
                        func=mybir.ActivationFunctionType.Tanh,
                    )
                    nc.gpsimd.tensor_tensor(
                        out=z01[:, :, t, :], in0=sig[:, 4:6], in1=th[:],
                        op=mybir.AluOpType.mult,
                    )

                def attention(t0, t1):
                    sb = t1 - t0
                    # all scores of the sblock into one psum tile, one Exp
                    ps_sall = aps1.tile([S, BL * 16], f32, tag="misc")
                    for b in range(BL):
                        for k in range(2):
                            nc.tensor.matmul(
                                ps_sall[:, b * sb : b * sb + sb],
                                encT_sb[:, k, b, :],
                                z01[:, k, t0:t1, b],
                                start=(k == 0),
                                stop=(k == 1),
                            )
                    nc.scalar.activation(
                        out=expsc[:, :, :sb],
                        in_=ps_sall[:, : BL * sb].rearrange(
                            "s (b t) -> s b t", b=BL
                        ),
                        func=mybir.ActivationFunctionType.Exp,
                        scale=inv_sqrt_h,
                    )
                    for b in range(BL):
                        nc.gpsimd.tensor_scalar(
                            out=expsc[:, b, :sb], in0=expsc[:, b, :sb],
                            scalar1=mask01[:, b : b + 1], scalar2=None,
                            op0=mybir.AluOpType.mult,
                        )
                    # softmax denominators for all (b, t'): [1, BL*sb]
                    ps_d = aps1.tile([1, BL * 16], f32, tag="misc")
                    nc.tensor.matmul(
                        ps_d[:, : BL * sb], ones_col[:], expsc[:, :, :sb],
                        start=True, stop=True,
                    )
                    nc.vector.reciprocal(
                        out=recip_sb[:, : BL * sb], in_=ps_d[:, : BL * sb]
                    )
                    # broadcast reciprocals to all 128 partitions
                    for ch in range((BL * sb + 127) // 128):
                        c0 = ch * 128
                        cn = min(128, BL * sb - c0)
                        ps_bc = aps1.tile([128, 128], f32, tag="misc")
                        nc.tensor.matmul(
                            ps_bc[:, :cn], ones_row[:],
                            recip_sb[:, c0 : c0 + cn],
                            start=True, stop=True,
                        )
                        nc.vector.tensor_copy(
                            out=bc_sb[:, c0 : c0 + cn], in_=ps_bc[:, :cn]
                        )
                    # ctx per (b, j), normalized into z23
                    for b in range(BL):
                        for j in range(2):
                            ps_c = aps.tile([128, 16], f32, tag="sc")
                            nc.tensor.matmul(
                                ps_c[:, :sb],
                                enc_sb[:, b, j * 128 : (j + 1) * 128],
                                expsc[:, b, :sb],
                                start=True, stop=True,
                            )
                            nc.vector.tensor_tensor(
                                out=z23[:, j, t0:t1, b],
                                in0=ps_c[:, :sb],
                                in1=bc_sb[:, b * sb : (b + 1) * sb],
                                op=mybir.AluOpType.mult,
                            )

                def gemm_pair(g, m):
                    n0 = g * GCOL
                    ps_o = ops.tile([128, 2, 512], f32, tag="po")
                    for k in range(4):
                        for n in range(2):
                            nc.tensor.matmul(
                                ps_o[:, n, :500],
                                zt[k][:, m * 128 : (m + 1) * 128],
                                wlin_sb[:, k, n0 + n * 500 : n0 + (n + 1) * 500],
                                start=(k == 0),
                                stop=(k == 3),
                            )
                    o_sb = osp.tile([128, GCOL], bf16, tag="osb")
                    pi = state["pair"]
                    if pi % 2 == 0:
                        nc.vector.tensor_copy(
                            out=o_sb[:].rearrange("p (a b) -> p a b", b=500),
                            in_=ps_o[:, :, :500],
                        )
                    else:
                        nc.scalar.copy(
                            out=o_sb[:].rearrange("p (a b) -> p a b", b=500),
                            in_=ps_o[:, :, :500],
                        )
                    # scatter to b-major rows: row = b*T + 8m + t_loc
                    dst = bass.AP(
                        tensor=out_d,
                        offset=(TPB * m) * VL + n0,
                        ap=[[VL, TPB], [T * VL, BL], [1, GCOL]],
                    )
                    dma_eng = [nc.gpsimd, nc.sync][pi % 2]
                    dma_eng.dma_start(out=dst, in_=o_sb[:])
                    state["pair"] = pi + 1

                def drain(nmax):
                    n = 0
                    while pendq and n < nmax:
                        gemm_pair(*pendq.pop(0))
                        n += 1

                sb_iter = iter(SBLOCKS)
                next_sb = next(sb_iter)
                for i in range(NMT):
                    for t in range(i * TPB, (i + 1) * TPB):
                        lstm_step(t)
                        drain(2 + (len(pendq) > 8) + (len(pendq) > 16))
                    while next_sb is not None and next_sb[2] == i:
                        attention(next_sb[0], next_sb[1])
                        next_sb = next(sb_iter, None)
                    new_m = [m for m in range(NMT) if MREADY[m] == i]
                    new_g = [g for g in range(NG) if ARRIVE[g] == i]
                    for g in new_g:
                        for m in ready_m:
                            pendq.append((g, m))
                    for m in new_m:
                        for g in arrived + new_g:
                            pendq.append((g, m))
                    arrived += new_g
                    ready_m += new_m
                drain(10**9)

    nc.compile()
    return nc


def _prep_inputs(inputs):
    bf = ml_dtypes.bfloat16
    target = np.asarray(inputs["target_tensor"])
    enc = np.asarray(inputs["encoder_outputs"], dtype=np.float32)
    lens = np.asarray(inputs["encoder_seq_lens"])
    h0 = np.asarray(inputs["h0"], dtype=np.float32)
    c0 = np.asarray(inputs["c0"], dtype=np.float32)
    emb = np.ascontiguousarray(np.asarray(inputs["emb"], dtype=np.float32))
    W_ih = np.asarray(inputs["W_ih"], dtype=np.float32)
    W_hh = np.asarray(inputs["W_hh"], dtype=np.float32)
    bias = (
        np.asarray(inputs["b_ih"], dtype=np.float32)
        + np.asarray(inputs["b_hh"], dtype=np.float32)
    )
    # permute gate order (i, f, g, o) -> (i, f, o, g)
    perm = np.concatenate(
        [np.arange(0, 2 * H), np.arange(3 * H, 4 * H), np.arange(2 * H, 3 * H)]
    )
    W_ih = W_ih[perm]
    W_hh = W_hh[perm]
    bias = bias[perm]
    W_lin = np.asarray(inputs["W_lin"], dtype=np.float32)

    wihT = np.ascontiguousarray(W_ih.T.astype(bf))                # (E, 4H)
    whhT = np.ascontiguousarray(W_hh.T.reshape(2, 128, G4H).astype(bf))
    biasT = np.ascontiguousarray(bias.reshape(NCH, 128).T)        # (128, NCH)
    wlinT_full = W_lin.T.astype(bf)                               # (512, V)

    in_maps = []
    for i in range(NCORES):
        bg = i % NBG
        vh = i // NBG
        sl = slice(bg * BL, (bg + 1) * BL)
        vsl = slice(vh * VL, (vh + 1) * VL)
        tok = np.ascontiguousarray(
            target[sl].T.reshape(BT, 1).astype(np.int32)
        )  # t-major
        enc_i = enc[sl]                                           # (BL, S, H)
        enc_sbh = np.ascontiguousarray(enc_i.transpose(1, 0, 2).astype(bf))
        encT = np.ascontiguousarray(
            enc_i.transpose(2, 0, 1).reshape(2, 128, BL, S).astype(bf)
        )
        h0T = np.ascontiguousarray(h0[sl].T.reshape(2, 128, BL).transpose(1, 0, 2))
        c0T = np.ascontiguousarray(c0[sl].T.reshape(2, 128, BL).transpose(1, 0, 2))
        wlinT = np.ascontiguousarray(wlinT_full[:, vsl].reshape(4, 128, VL))
        in_maps.append(
            {
                "tok": tok,
                "emb": emb,
                "enc": enc_sbh,
                "encT": encT,
                "h0T": h0T,
                "c0T": c0T,
                "lens": np.ascontiguousarray(lens[sl].astype(np.int32)),
                "biasT": biasT,
                "wihT": wihT,
                "whhT": whhT,
                "wlinT": wlinT,
            }
        )
    return in_maps


LAST_RESULTS = None


def _install_ntff_shim():
    import sys
    import types

    try:
        from antenv.axon_hooks import get_axon_ntff_profile_hook  # noqa: F401

        return
    except ImportError:
        pass
    try:
        from trn_agent_boot.trn_boot import _ntff_profile_via_ctypes

        hook = _ntff_profile_via_ctypes("/opt/axon/libaxon_pjrt.so")
        m = types.ModuleType("antenv.axon_hooks")
        m.get_axon_ntff_profile_hook = lambda: hook
        m.set_axon_ntff_profile_hook = lambda h: None
        sys.modules["antenv.axon_hooks"] = m
    except Exception:
        pass


def kernel(**inputs):
    global LAST_RESULTS
    _install_ntff_shim()
    if "nc" not in _CACHE:
        _CACHE["nc"] = _build()
    nc = _CACHE["nc"]
    in_maps = _prep_inputs(inputs)
    res = run_bass_kernel_spmd(nc, in_maps, core_ids=list(range(NCORES)))
    LAST_RESULTS = res
    b_lin = np.asarray(inputs["b_lin"], dtype=np.float32)
    out = np.empty((B, T, V), dtype=np.float32)
    for i in range(NCORES):
        bg = i % NBG
        vh = i // NBG
        vsl = slice(vh * VL, (vh + 1) * VL)
        # logits rows are b-major (row = b*T + t) thanks to the scatter DMA
        out[bg * BL : (bg + 1) * BL, :, vsl] = (
            np.asarray(res.results[i]["logits"]).astype(np.float32).reshape(BL, T, VL)
            + b_lin[vsl]
        )
    return out
